# revision 1
# baseline (speedup 1.0000x reference)
"""BatchBlur: depthwise 15x15 conv with per-sample kernels, reflection pad 7.

x: (32, 3, 512, 512) f32, kernel: (32, 15, 15) f32 -> out (32, 3, 512, 512) f32.

Strategy: pure data parallel over batch, 4 samples per core on 8 cores.
Host: reflection-pad x to (., 526, 526) and build banded "vertical" matrices
A[k, s, dx, m] = kern[s, k-m, dx] (0 <= k-m < 15).
Device: per 128-row strip of each padded channel image, accumulate 15 matmuls
in PSUM (one per horizontal tap dx):
  out[m, n] += sum_k A_dx[k, m] * xp[r0+k, n+dx]
which realizes the full 2D conv (vertical taps inside the band matrix,
horizontal taps via rhs free-dim offsets). float32r matmuls run at full PE
rate (1 cycle/row) with ~1e-4 relative error.
"""
import os
import sys

for _p in ("/opt/trn_rl_repo", "/root/.axon_site/_ro/trn_rl_repo"):
    if _p not in sys.path and os.path.isdir(_p):
        sys.path.insert(0, _p)

import numpy as np

import concourse.mybir as mybir
import concourse.tile as tile
from concourse import bacc
from concourse.bass_utils import run_bass_kernel_spmd

L = 15           # blur kernel size
P = L // 2       # reflection pad
B, C, H, W = 32, 3, 512, 512
N_CORES = 8
BS = B // N_CORES            # samples per core
NIMG = BS * C                # channel images per core
HP, WP = H + 2 * P, W + 2 * P  # 526
M_STRIP = 128 - (L - 1)      # 114 output rows per full strip
N_STRIPS = -(-H // M_STRIP)  # 5

F32 = mybir.dt.float32
F32R = mybir.dt.float32r

_program_cache = None


def _build_program():
    nc = bacc.Bacc("TRN2", target_bir_lowering=False, debug=False)
    xp_d = nc.dram_tensor("xp", [NIMG, HP, WP], F32R, kind="ExternalInput").ap()
    a_d = nc.dram_tensor("a", [128, BS, L, M_STRIP], F32R, kind="ExternalInput").ap()
    out_d = nc.dram_tensor("out", [NIMG, H, W], F32, kind="ExternalOutput").ap()

    with tile.TileContext(nc) as tc:
        with (
            tc.tile_pool(name="aconst", bufs=1) as apool,
            tc.tile_pool(name="xin", bufs=4) as xpool,
            tc.tile_pool(name="oout", bufs=4) as opool,
            tc.tile_pool(name="psum", bufs=4, space="PSUM") as psum,
        ):
            a_t = apool.tile([128, BS, L, M_STRIP], F32R)
            nc.sync.dma_start(out=a_t[:], in_=a_d[:])

            for img in range(NIMG):
                smp = img // C
                for s in range(N_STRIPS):
                    r0 = s * M_STRIP
                    m_s = min(M_STRIP, H - r0)   # output rows this strip
                    k_s = m_s + L - 1            # input rows this strip

                    xp_t = xpool.tile([128, WP], F32R)
                    nc.sync.dma_start(
                        out=xp_t[:k_s], in_=xp_d[img, r0:r0 + k_s, :]
                    )
                    acc = psum.tile([M_STRIP, W], F32)
                    for dx in range(L):
                        nc.tensor.matmul(
                            acc[:m_s],
                            a_t[:k_s, smp, dx, :m_s],
                            xp_t[:k_s, dx:dx + W],
                            start=(dx == 0),
                            stop=(dx == L - 1),
                        )
                    o_t = opool.tile([M_STRIP, W], F32)
                    nc.vector.tensor_copy(out=o_t[:m_s], in_=acc[:m_s])
                    nc.sync.dma_start(
                        out=out_d[img, r0:r0 + m_s, :], in_=o_t[:m_s]
                    )
    nc.compile()
    return nc


def kernel(x: np.ndarray, kernel: np.ndarray) -> np.ndarray:
    global _program_cache
    x = np.asarray(x, dtype=np.float32)
    kern = np.asarray(kernel, dtype=np.float32)

    # host-side reflection pad
    xp = np.pad(x, ((0, 0), (0, 0), (P, P), (P, P)), mode="reflect")
    xp = np.ascontiguousarray(xp.reshape(B * C, HP, WP))

    # band matrices: a_all[k, s, dx, m] = kern[s, k-m, dx] for 0 <= k-m < L
    a_all = np.zeros((128, B, L, M_STRIP), dtype=np.float32)
    m_idx = np.arange(M_STRIP)
    for dy in range(L):
        a_all[m_idx + dy, :, :, m_idx] = kern[:, dy, :]

    if _program_cache is None:
        _program_cache = _build_program()
    nc = _program_cache

    in_maps = []
    for c in range(N_CORES):
        in_maps.append({
            "xp": xp[c * NIMG:(c + 1) * NIMG],
            "a": np.ascontiguousarray(a_all[:, c * BS:(c + 1) * BS]),
        })
    res = run_bass_kernel_spmd(nc, in_maps, core_ids=list(range(N_CORES)))
    out = np.concatenate([r["out"] for r in res.results], axis=0)
    return out.reshape(B, C, H, W)
